# revision 17
# baseline (speedup 1.0000x reference)
"""Expert-parallel MoE grouped-experts kernel for 8 trn2 NeuronCores.

Contract: kernel(**inputs) takes FULL unsharded inputs, returns FULL output.

Strategy (expert-parallel, variant "bf16x" default):
  - Host: sort token-expert assignments by expert. Experts are ranked by
    size (desc) and dealt round-robin to cores: core c gets experts ranked
    c, 8+c, 16+c, 24+c. The SPMD program has 4 static block sizes
    Ce[b] = max size within rank octile b (so padding is the octile spread,
    ~1-2%, instead of global-max padding ~12%).
  - Device (SPMD x8, one static program): per block, per 1024-row chunk:
    grouped GEMMs with weights stationary and rows moving everywhere:
      pg/pu[i-tile, rows] = sum_hc gw/uw[hc, i-tile].T @ xs[hc, rows]
      hm[it] = silu(pg) * pu                       (bf16)
      oT[h-tile, rows] = sum_ic dw[ic, h-tile].T @ hm[ic]
    All matmuls bf16 at 1 cy/row, fp32 PSUM accumulation; LDWEIGHTS (128
    cols, FWL) hides behind 512-col matmuls.
  - Host: transpose oT back, scale by routing weights, scatter-add.

Older variants kept for fallback: "tf32" (f32r), "bf16" (uniform-Ce).
"""
import os
import sys

if "/opt/trn_rl_repo" not in sys.path:
    sys.path.insert(0, "/opt/trn_rl_repo")

import math
import numpy as np
import ml_dtypes

B, S, H, I, E, K = 4, 4096, 2048, 1024, 32, 4
N = B * S
NCORES = 8
EPC = E // NCORES  # experts per core (blocks)
HC = H // 128      # 16 h-chunks (gate/up contraction tiles)
IC = I // 128      # 8 i-chunks (down contraction tiles)
IT = I // 128      # 8 i-tiles
HT = H // 128      # 16 h-tiles (down output tiles)
CHUNK = 1152       # xs rows resident per chunk (tile allocation size)


def _even_split(total: int, cap: int, align: int = 16):
    """Split `total` into near-equal pieces of <= cap, multiples of `align`
    (except possibly the last). Avoids tiny tail pieces."""
    if total <= 0:
        return []
    n = math.ceil(total / cap)
    step = math.ceil(total / n / align) * align
    out = []
    r = 0
    while r < total:
        out.append((r, min(step, total - r)))
        r += out[-1][1]
    return out

VARIANT = os.environ.get("MOE_VARIANT", "bf16x")
_SIM_ACT = os.environ.get("MOE_SIM_ACT", "")  # e.g. "Sigmoid" for CoreSim checks

_LAST_RESULTS = None  # BassKernelResults of the most recent run (for test.py)


def _round_tf32(x: np.ndarray) -> np.ndarray:
    """Round f32 to tf32 (10-bit mantissa), round-to-nearest-even."""
    u = np.ascontiguousarray(x, dtype=np.float32).view(np.uint32).astype(np.uint64)
    u = u + 0x0FFF + ((u >> 13) & 1)
    u = (u & np.uint64(0xFFFFE000)).astype(np.uint32)
    return u.view(np.float32)


def _build_bf16x(ce_list):
    """bf16 exact-ish-size kernel: 4 expert blocks with static sizes ce_list."""
    import concourse.tile as tile
    import concourse.mybir as mybir
    from concourse import bacc

    bf16 = mybir.dt.bfloat16
    f32 = mybir.dt.float32

    CT = int(sum(ce_list))
    # flat chunk list: (block, c0, cl); xsC is chunk-major contiguous
    chunk_list = []
    for b in range(EPC):
        for c0, cl in _even_split(int(ce_list[b]), CHUNK):
            chunk_list.append((b, c0, cl))
    NCH = len(chunk_list)

    nc = bacc.Bacc("TRN2", target_bir_lowering=False, debug=False)

    xsC = nc.dram_tensor("xsC", [NCH, 128, HC, CHUNK], bf16, kind="ExternalInput")
    gwP = nc.dram_tensor("gwP", [EPC, IT, 128, HC, 128], bf16, kind="ExternalInput")
    uwP = nc.dram_tensor("uwP", [EPC, IT, 128, HC, 128], bf16, kind="ExternalInput")
    dwP = nc.dram_tensor("dwP", [EPC, HT, 128, IC, 128], bf16, kind="ExternalInput")
    oT = nc.dram_tensor("oT", [H, CT], bf16, kind="ExternalOutput")

    with tile.TileContext(nc) as tc:
        with (
            tc.tile_pool(name="xs", bufs=3) as xsp,
            tc.tile_pool(name="hm", bufs=16) as hmp,
            tc.tile_pool(name="wg", bufs=2) as wgp,
            tc.tile_pool(name="wu", bufs=2) as wup,
            tc.tile_pool(name="wd", bufs=2) as wdp,
            tc.tile_pool(name="sg", bufs=2) as sgp,
            tc.tile_pool(name="ot", bufs=4) as otp,
            tc.tile_pool(name="psum", bufs=8, space="PSUM") as psp,
        ):
            bases = [int(sum(ce_list[:b])) for b in range(EPC)]

            def load_chunk(ci):
                t = xsp.tile([128, HC, CHUNK], bf16, tag="xs", name="xst")
                with tc.high_priority():
                    nc.sync.dma_start(t[:], xsC.ap()[ci])
                return t

            pending = [load_chunk(0)]
            if NCH > 1:
                pending.append(load_chunk(1))
            for ci, (b, c0, cl) in enumerate(chunk_list):
                base = bases[b]
                if ci + 2 < NCH:
                    pending.append(load_chunk(ci + 2))
                xst_full = pending.pop(0)
                xst = xst_full[:, :, :cl]
                if True:
                    slices = _even_split(cl, 512)
                    hms = []
                    for it in range(IT):
                        gw = wgp.tile([128, HC, 128], bf16, tag="gw")
                        nc.gpsimd.dma_start(gw[:], gwP.ap()[b, it])
                        uw = wup.tile([128, HC, 128], bf16, tag="uw")
                        nc.gpsimd.dma_start(uw[:], uwP.ap()[b, it])
                        hm_full = hmp.tile([128, CHUNK], bf16, tag="hm")
                        hm = hm_full[:, :cl]
                        hms.append(hm)
                        # one pg/pu PSUM bank per slice; stationary weight
                        # tiles are reused across all slices (slice-inner)
                        pgs = []
                        for r0, rl in slices:
                            pg_full = psp.tile([128, 512], f32, tag="ps")
                            pgs.append(pg_full[:, :rl])
                        for hc in range(HC):
                            for si, (r0, rl) in enumerate(slices):
                                nc.tensor.matmul(
                                    pgs[si], gw[:, hc, :], xst[:, hc, r0 : r0 + rl],
                                    start=(hc == 0), stop=(hc == HC - 1),
                                )
                        pus = []
                        for r0, rl in slices:
                            pu_full = psp.tile([128, 512], f32, tag="ps")
                            pus.append(pu_full[:, :rl])
                        for hc in range(HC):
                            for si, (r0, rl) in enumerate(slices):
                                nc.tensor.matmul(
                                    pus[si], uw[:, hc, :], xst[:, hc, r0 : r0 + rl],
                                    start=(hc == 0), stop=(hc == HC - 1),
                                )
                        act = getattr(
                            mybir.ActivationFunctionType, _SIM_ACT or "Silu"
                        )
                        for si, (r0, rl) in enumerate(slices):
                            sg_full = sgp.tile([128, 512], f32, tag="sg")
                            sg = sg_full[:, :rl]
                            nc.scalar.activation(sg, pgs[si], act)
                            nc.vector.tensor_mul(hm[:, r0 : r0 + rl], sg, pus[si])

                    for ht in range(HT):
                        dwt = wdp.tile([128, IC, 128], bf16, tag="dw")
                        nc.gpsimd.dma_start(dwt[:], dwP.ap()[b, ht])
                        # slice pairs: <=2 po banks live at a time
                        for p0 in range(0, len(slices), 2):
                            spair = slices[p0 : p0 + 2]
                            pos = []
                            for r0, rl in spair:
                                po_full = psp.tile([128, 512], f32, tag="ps")
                                pos.append(po_full[:, :rl])
                            for ic in range(IC):
                                for si, (r0, rl) in enumerate(spair):
                                    nc.tensor.matmul(
                                        pos[si], dwt[:, ic, :],
                                        hms[ic][:, r0 : r0 + rl],
                                        start=(ic == 0), stop=(ic == IC - 1),
                                    )
                            for si, (r0, rl) in enumerate(spair):
                                ot = otp.tile([128, 512], bf16, tag="ot")
                                ott = ot[:, :rl]
                                nc.vector.tensor_copy(ott, pos[si])
                                nc.scalar.dma_start(
                                    oT.ap()[
                                        ht * 128 : (ht + 1) * 128,
                                        base + c0 + r0 : base + c0 + r0 + rl,
                                    ],
                                    ott,
                                )
    nc.compile()
    return nc


def _pack_gu(w_bf):
    """(I, H) bf16 -> [IT, 128p(h%128), HC, 128(i%128)] contiguous."""
    w4 = w_bf.reshape(IT, 128, HC, 128)  # (it, il, hc, p)
    return np.ascontiguousarray(w4.transpose(0, 3, 2, 1))


def _pack_d(w_bf):
    """(H, I) bf16 -> [HT, 128p(i%128), IC, 128(h%128)] contiguous."""
    w4 = w_bf.reshape(HT, 128, IC, 128)  # (ht, hl, ic, p)
    return np.ascontiguousarray(w4.transpose(0, 3, 2, 1))


def _kernel_bf16x(hidden_states, gate_weight, up_weight, down_weight, topk_idx, topk_weight):
    global _LAST_RESULTS
    from concourse.bass_utils import run_bass_kernel_spmd

    bf16 = ml_dtypes.bfloat16

    x = np.ascontiguousarray(hidden_states, dtype=np.float32).reshape(N, H)
    flat_expert = np.asarray(topk_idx).reshape(-1).astype(np.int64)
    flat_weight = np.asarray(topk_weight).reshape(-1).astype(np.float32)

    perm = np.argsort(flat_expert, kind="stable")
    tok_sorted = np.repeat(np.arange(N), K)[perm]
    sizes = np.bincount(flat_expert, minlength=E)
    offs = np.concatenate([[0], np.cumsum(sizes)])

    order = np.argsort(-sizes, kind="stable")  # experts by size desc
    # block b on core c processes expert order[b*NCORES + c]
    ce_list = [
        int(math.ceil(max(int(sizes[order[b * NCORES]]), 16) / 16) * 16)
        for b in range(EPC)
    ]
    bases = np.concatenate([[0], np.cumsum(ce_list)])
    CT = int(bases[-1])
    chunk_list = []
    for b in range(EPC):
        for c0, cl in _even_split(int(ce_list[b]), CHUNK):
            chunk_list.append((b, c0, cl))
    NCH = len(chunk_list)

    xs_all = x[tok_sorted].astype(bf16)  # (N*K, H) expert-contiguous rows
    gw_bf = np.asarray(gate_weight).astype(bf16)
    uw_bf = np.asarray(up_weight).astype(bf16)
    dw_bf = np.asarray(down_weight).astype(bf16)

    in_maps = []
    for c in range(NCORES):
        xsC_m = np.zeros((NCH, 128, HC, CHUNK), dtype=bf16)
        gw_l, uw_l, dw_l = [], [], []
        for ci, (b, c0, cl) in enumerate(chunk_list):
            e = int(order[b * NCORES + c])
            n_e = int(sizes[e])
            valid = max(0, min(cl, n_e - c0))
            if valid > 0:
                blk = xs_all[offs[e] + c0 : offs[e] + c0 + valid].T  # (H, valid)
                xsC_m[ci, :, :, :valid] = blk.reshape(HC, 128, valid).transpose(
                    1, 0, 2
                )
        for b in range(EPC):
            e = int(order[b * NCORES + c])
            gw_l.append(_pack_gu(gw_bf[e]))
            uw_l.append(_pack_gu(uw_bf[e]))
            dw_l.append(_pack_d(dw_bf[e]))
        in_maps.append(
            {
                "xsC": xsC_m,
                "gwP": np.stack(gw_l),
                "uwP": np.stack(uw_l),
                "dwP": np.stack(dw_l),
            }
        )

    nc = _build_bf16x(ce_list)
    res = run_bass_kernel_spmd(nc, in_maps, core_ids=list(range(NCORES)))
    _LAST_RESULTS = res

    # combine: weighted scatter-add back to token order
    o_sorted = np.empty((N * K, H), dtype=np.float32)
    for c in range(NCORES):
        om = res.results[c]["oT"]  # (H, CT) bf16
        for b in range(EPC):
            e = int(order[b * NCORES + c])
            n_e = int(sizes[e])
            o_sorted[offs[e] : offs[e + 1]] = (
                om[:, bases[b] : bases[b] + n_e].T.astype(np.float32)
            )
    o_sorted *= flat_weight[perm][:, None]
    o_orig = np.empty_like(o_sorted)
    o_orig[perm] = o_sorted
    y = o_orig.reshape(N, K, H).sum(axis=1)
    return y.reshape(B, S, H).astype(np.float32)


# ---------------------------------------------------------------------------
# Legacy variants (tf32 / bf16 uniform-Ce), kept as fallback.
# ---------------------------------------------------------------------------


def _row_chunks(ce: int, chunk: int = 1152):
    out = []
    r = 0
    while r < ce:
        out.append((r, min(chunk, ce - r)))
        r += out[-1][1]
    return out


def _move_slices(length: int):
    out = []
    r = 0
    while r < length:
        rem = length - r
        if rem <= 512:
            s = rem
        elif rem - 384 >= 256:
            s = 384
        else:
            s = 512
        out.append((r, s))
        r += s
    return out


def _build_tf32(CT: int):
    import concourse.tile as tile
    import concourse.mybir as mybir
    from concourse import bacc

    f32 = mybir.dt.float32
    f32r = mybir.dt.float32r

    nc = bacc.Bacc("TRN2", target_bir_lowering=False, debug=False)

    xsT = nc.dram_tensor("xsT", [H, CT], f32r, kind="ExternalInput")
    gwP = nc.dram_tensor("gwP", [EPC, IT, 128, HC, 128], f32r, kind="ExternalInput")
    uwP = nc.dram_tensor("uwP", [EPC, IT, 128, HC, 128], f32r, kind="ExternalInput")
    dwT = nc.dram_tensor("dwT", [EPC, I, H], f32r, kind="ExternalInput")
    o = nc.dram_tensor("o", [CT, H], f32, kind="ExternalOutput")

    Ce = CT // EPC
    chunks = _row_chunks(Ce)

    with tile.TileContext(nc) as tc:
        with (
            tc.tile_pool(name="xs", bufs=1) as xsp,
            tc.tile_pool(name="wg", bufs=2) as wg,
            tc.tile_pool(name="wu", bufs=2) as wu,
            tc.tile_pool(name="wd", bufs=2) as wd,
            tc.tile_pool(name="hm", bufs=1) as hmp,
            tc.tile_pool(name="sg", bufs=2) as sgp,
            tc.tile_pool(name="ost", bufs=4) as ostp,
            tc.tile_pool(name="psum", bufs=6, space="PSUM") as psp,
        ):
            for e in range(EPC):
                for c0, cl in chunks:
                    base = e * Ce + c0
                    xst_full = xsp.tile([128, HC, 1152], f32r, tag="xs")
                    xst = xst_full[:, :, :cl]
                    for hcb in range(HC):
                        nc.sync.dma_start(
                            xst[:, hcb, :],
                            xsT.ap()[hcb * 128 : (hcb + 1) * 128, base : base + cl],
                        )
                    hm_full = hmp.tile([128, IC, 1152], f32r, tag="hm")
                    hm = hm_full[:, :, :cl]
                    for it in range(IT):
                        gw = wg.tile([128, HC, 128], f32r, tag="gw")
                        nc.gpsimd.dma_start(gw[:], gwP.ap()[e, it])
                        uw = wu.tile([128, HC, 128], f32r, tag="uw")
                        nc.gpsimd.dma_start(uw[:], uwP.ap()[e, it])
                        for r0, rl in _move_slices(cl):
                            pg_full = psp.tile([128, 512], f32, tag="ps")
                            pu_full = psp.tile([128, 512], f32, tag="ps")
                            pg = pg_full[:, :rl]
                            pu = pu_full[:, :rl]
                            for hc in range(HC):
                                nc.tensor.matmul(
                                    pg[:], gw[:, hc, :], xst[:, hc, r0 : r0 + rl],
                                    start=(hc == 0), stop=(hc == HC - 1),
                                )
                            for hc in range(HC):
                                nc.tensor.matmul(
                                    pu[:], uw[:, hc, :], xst[:, hc, r0 : r0 + rl],
                                    start=(hc == 0), stop=(hc == HC - 1),
                                )
                            sg_full = sgp.tile([128, 512], f32, tag="sg")
                            sg = sg_full[:, :rl]
                            nc.scalar.activation(
                                sg[:], pg[:], mybir.ActivationFunctionType.Silu
                            )
                            nc.vector.tensor_mul(hm[:, it, r0 : r0 + rl], sg[:], pu[:])

                    for hs in range(H // 512):
                        dw = wd.tile([128, IC, 512], f32r, tag="dw")
                        nc.sync.dma_start(
                            dw[:],
                            dwT.ap()[e][:, hs * 512 : (hs + 1) * 512].rearrange(
                                "(c p) h -> p c h", p=128
                            ),
                        )
                        for rt in range(cl // 128):
                            po = psp.tile([128, 512], f32, tag="ps")
                            for ic in range(IC):
                                nc.tensor.matmul(
                                    po[:], hm[:, ic, rt * 128 : (rt + 1) * 128],
                                    dw[:, ic, :], start=(ic == 0), stop=(ic == IC - 1),
                                )
                            ot = ostp.tile([128, 512], f32, tag="o")
                            nc.vector.tensor_copy(ot[:], po[:])
                            nc.scalar.dma_start(
                                o.ap()[
                                    base + rt * 128 : base + (rt + 1) * 128,
                                    hs * 512 : (hs + 1) * 512,
                                ],
                                ot[:],
                            )
    nc.compile()
    return nc


def _kernel_tf32(hidden_states, gate_weight, up_weight, down_weight, topk_idx, topk_weight):
    global _LAST_RESULTS
    from concourse.bass_utils import run_bass_kernel_spmd

    x = np.ascontiguousarray(hidden_states, dtype=np.float32).reshape(N, H)
    flat_expert = np.asarray(topk_idx).reshape(-1).astype(np.int64)
    flat_weight = np.asarray(topk_weight).reshape(-1).astype(np.float32)

    perm = np.argsort(flat_expert, kind="stable")
    tok_sorted = np.repeat(np.arange(N), K)[perm]
    sizes = np.bincount(flat_expert, minlength=E)
    offs = np.concatenate([[0], np.cumsum(sizes)])

    Ce = int(math.ceil(sizes.max() / 256) * 256)
    CT = EPC * Ce

    gw_all = np.asarray(gate_weight, dtype=np.float32)
    uw_all = np.asarray(up_weight, dtype=np.float32)
    dw_all = np.asarray(down_weight, dtype=np.float32)

    in_maps = []
    for m in range(NCORES):
        sl = slice(m * EPC, (m + 1) * EPC)
        xsT_m = np.zeros((H, CT), dtype=np.float32)
        for el in range(EPC):
            ex = m * EPC + el
            ids = tok_sorted[offs[ex] : offs[ex + 1]]
            xsT_m[:, el * Ce : el * Ce + len(ids)] = _round_tf32(x[ids]).T

        def pack_gu(w):  # w: (EPC, I, H)
            w4 = w.reshape(EPC, IT, 128, HC, 128)  # (e, it, il, hc, p)
            return _round_tf32(np.ascontiguousarray(w4.transpose(0, 1, 4, 3, 2)))

        in_maps.append(
            {
                "xsT": xsT_m,
                "gwP": pack_gu(gw_all[sl]),
                "uwP": pack_gu(uw_all[sl]),
                "dwT": _round_tf32(
                    np.ascontiguousarray(dw_all[sl].transpose(0, 2, 1))
                ),
            }
        )

    nc = _build_tf32(CT)
    res = run_bass_kernel_spmd(nc, in_maps, core_ids=list(range(NCORES)))
    _LAST_RESULTS = res

    o_sorted = np.empty((N * K, H), dtype=np.float32)
    for m in range(NCORES):
        om = res.results[m]["o"]
        for el in range(EPC):
            ex = m * EPC + el
            n_e = offs[ex + 1] - offs[ex]
            o_sorted[offs[ex] : offs[ex + 1]] = om[el * Ce : el * Ce + n_e]
    o_sorted *= flat_weight[perm][:, None]
    o_orig = np.empty_like(o_sorted)
    o_orig[perm] = o_sorted
    y = o_orig.reshape(N, K, H).sum(axis=1)
    return y.reshape(B, S, H).astype(np.float32)


def kernel(hidden_states, gate_weight, up_weight, down_weight, topk_idx, topk_weight):
    if VARIANT == "tf32":
        return _kernel_tf32(
            hidden_states, gate_weight, up_weight, down_weight, topk_idx, topk_weight
        )
    return _kernel_bf16x(
        hidden_states, gate_weight, up_weight, down_weight, topk_idx, topk_weight
    )


# revision 18
# speedup vs baseline: 1.1607x; 1.1607x over previous
"""Expert-parallel MoE grouped-experts kernel for 8 trn2 NeuronCores.

Contract: kernel(**inputs) takes FULL unsharded inputs, returns FULL output.

Strategy (expert-parallel, variant "bf16x" default):
  - Host: sort token-expert assignments by expert. Experts are ranked by
    size (desc) and dealt round-robin to cores: core c gets experts ranked
    c, 8+c, 16+c, 24+c. The SPMD program has 4 static block sizes
    Ce[b] = max size within rank octile b (so padding is the octile spread,
    ~1-2%, instead of global-max padding ~12%).
  - Device (SPMD x8, one static program): per block, per 1024-row chunk:
    grouped GEMMs with weights stationary and rows moving everywhere:
      pg/pu[i-tile, rows] = sum_hc gw/uw[hc, i-tile].T @ xs[hc, rows]
      hm[it] = silu(pg) * pu                       (bf16)
      oT[h-tile, rows] = sum_ic dw[ic, h-tile].T @ hm[ic]
    All matmuls bf16 at 1 cy/row, fp32 PSUM accumulation; LDWEIGHTS (128
    cols, FWL) hides behind 512-col matmuls.
  - Host: transpose oT back, scale by routing weights, scatter-add.

Older variants kept for fallback: "tf32" (f32r), "bf16" (uniform-Ce).
"""
import os
import sys

if "/opt/trn_rl_repo" not in sys.path:
    sys.path.insert(0, "/opt/trn_rl_repo")

import math
import numpy as np
import ml_dtypes

B, S, H, I, E, K = 4, 4096, 2048, 1024, 32, 4
N = B * S
NCORES = 8
EPC = E // NCORES  # experts per core (blocks)
HC = H // 128      # 16 h-chunks (gate/up contraction tiles)
IC = I // 128      # 8 i-chunks (down contraction tiles)
IT = I // 128      # 8 i-tiles
HT = H // 128      # 16 h-tiles (down output tiles)
CHUNK = 1152       # xs rows resident per chunk (tile allocation size)


def _even_split(total: int, cap: int, align: int = 16):
    """Split `total` into near-equal pieces of <= cap, multiples of `align`
    (except possibly the last). Avoids tiny tail pieces."""
    if total <= 0:
        return []
    n = math.ceil(total / cap)
    step = math.ceil(total / n / align) * align
    out = []
    r = 0
    while r < total:
        out.append((r, min(step, total - r)))
        r += out[-1][1]
    return out

VARIANT = os.environ.get("MOE_VARIANT", "bf16x")
_SIM_ACT = os.environ.get("MOE_SIM_ACT", "")  # e.g. "Sigmoid" for CoreSim checks

_LAST_RESULTS = None  # BassKernelResults of the most recent run (for test.py)


def _round_tf32(x: np.ndarray) -> np.ndarray:
    """Round f32 to tf32 (10-bit mantissa), round-to-nearest-even."""
    u = np.ascontiguousarray(x, dtype=np.float32).view(np.uint32).astype(np.uint64)
    u = u + 0x0FFF + ((u >> 13) & 1)
    u = (u & np.uint64(0xFFFFE000)).astype(np.uint32)
    return u.view(np.float32)


def _build_bf16x(ce_list):
    """bf16 exact-ish-size kernel: 4 expert blocks with static sizes ce_list."""
    import concourse.tile as tile
    import concourse.mybir as mybir
    from concourse import bacc

    bf16 = mybir.dt.bfloat16
    f32 = mybir.dt.float32

    CT = int(sum(ce_list))
    # flat chunk list: (block, c0, cl); xsC is chunk-major contiguous
    chunk_list = []
    for b in range(EPC):
        for c0, cl in _even_split(int(ce_list[b]), CHUNK):
            chunk_list.append((b, c0, cl))
    NCH = len(chunk_list)

    nc = bacc.Bacc("TRN2", target_bir_lowering=False, debug=False)

    xsC = nc.dram_tensor("xsC", [NCH, 128, HC, CHUNK], bf16, kind="ExternalInput")
    gwP = nc.dram_tensor("gwP", [EPC, IT, 128, HC, 128], bf16, kind="ExternalInput")
    uwP = nc.dram_tensor("uwP", [EPC, IT, 128, HC, 128], bf16, kind="ExternalInput")
    dwP = nc.dram_tensor("dwP", [EPC, HT, 128, IC, 128], bf16, kind="ExternalInput")
    oT = nc.dram_tensor("oT", [H, CT], bf16, kind="ExternalOutput")

    with tile.TileContext(nc) as tc:
        with (
            tc.tile_pool(name="xs", bufs=3) as xsp,
            tc.tile_pool(name="hm", bufs=16) as hmp,
            tc.tile_pool(name="wg", bufs=2) as wgp,
            tc.tile_pool(name="wu", bufs=2) as wup,
            tc.tile_pool(name="wd", bufs=2) as wdp,
            tc.tile_pool(name="sg", bufs=2) as sgp,
            tc.tile_pool(name="ot", bufs=4) as otp,
            tc.tile_pool(name="psum", bufs=8, space="PSUM") as psp,
        ):
            bases = [int(sum(ce_list[:b])) for b in range(EPC)]

            for ci, (b, c0, cl) in enumerate(chunk_list):
                base = bases[b]
                xst_full = xsp.tile([128, HC, CHUNK], bf16, tag="xs", name="xst")
                nc.sync.dma_start(xst_full[:], xsC.ap()[ci])
                xst = xst_full[:, :, :cl]
                if True:
                    slices = _even_split(cl, 512)
                    hms = []
                    for it in range(IT):
                        gw = wgp.tile([128, HC, 128], bf16, tag="gw")
                        nc.gpsimd.dma_start(gw[:], gwP.ap()[b, it])
                        uw = wup.tile([128, HC, 128], bf16, tag="uw")
                        nc.gpsimd.dma_start(uw[:], uwP.ap()[b, it])
                        hm_full = hmp.tile([128, CHUNK], bf16, tag="hm")
                        hm = hm_full[:, :cl]
                        hms.append(hm)
                        # one pg/pu PSUM bank per slice; stationary weight
                        # tiles are reused across all slices (slice-inner)
                        pgs = []
                        for r0, rl in slices:
                            pg_full = psp.tile([128, 512], f32, tag="ps")
                            pgs.append(pg_full[:, :rl])
                        for hc in range(HC):
                            for si, (r0, rl) in enumerate(slices):
                                nc.tensor.matmul(
                                    pgs[si], gw[:, hc, :], xst[:, hc, r0 : r0 + rl],
                                    start=(hc == 0), stop=(hc == HC - 1),
                                )
                        pus = []
                        for r0, rl in slices:
                            pu_full = psp.tile([128, 512], f32, tag="ps")
                            pus.append(pu_full[:, :rl])
                        for hc in range(HC):
                            for si, (r0, rl) in enumerate(slices):
                                nc.tensor.matmul(
                                    pus[si], uw[:, hc, :], xst[:, hc, r0 : r0 + rl],
                                    start=(hc == 0), stop=(hc == HC - 1),
                                )
                        act = getattr(
                            mybir.ActivationFunctionType, _SIM_ACT or "Silu"
                        )
                        for si, (r0, rl) in enumerate(slices):
                            sg_full = sgp.tile([128, 512], f32, tag="sg")
                            sg = sg_full[:, :rl]
                            nc.scalar.activation(sg, pgs[si], act)
                            nc.vector.tensor_mul(hm[:, r0 : r0 + rl], sg, pus[si])

                    for ht in range(HT):
                        dwt = wdp.tile([128, IC, 128], bf16, tag="dw")
                        nc.gpsimd.dma_start(dwt[:], dwP.ap()[b, ht])
                        # slice pairs: <=2 po banks live at a time
                        for p0 in range(0, len(slices), 2):
                            spair = slices[p0 : p0 + 2]
                            pos = []
                            for r0, rl in spair:
                                po_full = psp.tile([128, 512], f32, tag="ps")
                                pos.append(po_full[:, :rl])
                            for ic in range(IC):
                                for si, (r0, rl) in enumerate(spair):
                                    nc.tensor.matmul(
                                        pos[si], dwt[:, ic, :],
                                        hms[ic][:, r0 : r0 + rl],
                                        start=(ic == 0), stop=(ic == IC - 1),
                                    )
                            for si, (r0, rl) in enumerate(spair):
                                ot = otp.tile([128, 512], bf16, tag="ot")
                                ott = ot[:, :rl]
                                nc.vector.tensor_copy(ott, pos[si])
                                nc.scalar.dma_start(
                                    oT.ap()[
                                        ht * 128 : (ht + 1) * 128,
                                        base + c0 + r0 : base + c0 + r0 + rl,
                                    ],
                                    ott,
                                )
    nc.compile()
    return nc


def _pack_gu(w_bf):
    """(I, H) bf16 -> [IT, 128p(h%128), HC, 128(i%128)] contiguous."""
    w4 = w_bf.reshape(IT, 128, HC, 128)  # (it, il, hc, p)
    return np.ascontiguousarray(w4.transpose(0, 3, 2, 1))


def _pack_d(w_bf):
    """(H, I) bf16 -> [HT, 128p(i%128), IC, 128(h%128)] contiguous."""
    w4 = w_bf.reshape(HT, 128, IC, 128)  # (ht, hl, ic, p)
    return np.ascontiguousarray(w4.transpose(0, 3, 2, 1))


def _kernel_bf16x(hidden_states, gate_weight, up_weight, down_weight, topk_idx, topk_weight):
    global _LAST_RESULTS
    from concourse.bass_utils import run_bass_kernel_spmd

    bf16 = ml_dtypes.bfloat16

    x = np.ascontiguousarray(hidden_states, dtype=np.float32).reshape(N, H)
    flat_expert = np.asarray(topk_idx).reshape(-1).astype(np.int64)
    flat_weight = np.asarray(topk_weight).reshape(-1).astype(np.float32)

    perm = np.argsort(flat_expert, kind="stable")
    tok_sorted = np.repeat(np.arange(N), K)[perm]
    sizes = np.bincount(flat_expert, minlength=E)
    offs = np.concatenate([[0], np.cumsum(sizes)])

    order = np.argsort(-sizes, kind="stable")  # experts by size desc
    # block b on core c processes expert order[b*NCORES + c]
    ce_list = [
        int(math.ceil(max(int(sizes[order[b * NCORES]]), 16) / 16) * 16)
        for b in range(EPC)
    ]
    bases = np.concatenate([[0], np.cumsum(ce_list)])
    CT = int(bases[-1])
    chunk_list = []
    for b in range(EPC):
        for c0, cl in _even_split(int(ce_list[b]), CHUNK):
            chunk_list.append((b, c0, cl))
    NCH = len(chunk_list)

    xs_all = x[tok_sorted].astype(bf16)  # (N*K, H) expert-contiguous rows
    gw_bf = np.asarray(gate_weight).astype(bf16)
    uw_bf = np.asarray(up_weight).astype(bf16)
    dw_bf = np.asarray(down_weight).astype(bf16)

    in_maps = []
    for c in range(NCORES):
        xsC_m = np.zeros((NCH, 128, HC, CHUNK), dtype=bf16)
        gw_l, uw_l, dw_l = [], [], []
        for ci, (b, c0, cl) in enumerate(chunk_list):
            e = int(order[b * NCORES + c])
            n_e = int(sizes[e])
            valid = max(0, min(cl, n_e - c0))
            if valid > 0:
                blk = xs_all[offs[e] + c0 : offs[e] + c0 + valid].T  # (H, valid)
                xsC_m[ci, :, :, :valid] = blk.reshape(HC, 128, valid).transpose(
                    1, 0, 2
                )
        for b in range(EPC):
            e = int(order[b * NCORES + c])
            gw_l.append(_pack_gu(gw_bf[e]))
            uw_l.append(_pack_gu(uw_bf[e]))
            dw_l.append(_pack_d(dw_bf[e]))
        in_maps.append(
            {
                "xsC": xsC_m,
                "gwP": np.stack(gw_l),
                "uwP": np.stack(uw_l),
                "dwP": np.stack(dw_l),
            }
        )

    nc = _build_bf16x(ce_list)
    res = run_bass_kernel_spmd(nc, in_maps, core_ids=list(range(NCORES)))
    _LAST_RESULTS = res

    # combine: weighted scatter-add back to token order
    o_sorted = np.empty((N * K, H), dtype=np.float32)
    for c in range(NCORES):
        om = res.results[c]["oT"]  # (H, CT) bf16
        for b in range(EPC):
            e = int(order[b * NCORES + c])
            n_e = int(sizes[e])
            o_sorted[offs[e] : offs[e + 1]] = (
                om[:, bases[b] : bases[b] + n_e].T.astype(np.float32)
            )
    o_sorted *= flat_weight[perm][:, None]
    o_orig = np.empty_like(o_sorted)
    o_orig[perm] = o_sorted
    y = o_orig.reshape(N, K, H).sum(axis=1)
    return y.reshape(B, S, H).astype(np.float32)


# ---------------------------------------------------------------------------
# Legacy variants (tf32 / bf16 uniform-Ce), kept as fallback.
# ---------------------------------------------------------------------------


def _row_chunks(ce: int, chunk: int = 1152):
    out = []
    r = 0
    while r < ce:
        out.append((r, min(chunk, ce - r)))
        r += out[-1][1]
    return out


def _move_slices(length: int):
    out = []
    r = 0
    while r < length:
        rem = length - r
        if rem <= 512:
            s = rem
        elif rem - 384 >= 256:
            s = 384
        else:
            s = 512
        out.append((r, s))
        r += s
    return out


def _build_tf32(CT: int):
    import concourse.tile as tile
    import concourse.mybir as mybir
    from concourse import bacc

    f32 = mybir.dt.float32
    f32r = mybir.dt.float32r

    nc = bacc.Bacc("TRN2", target_bir_lowering=False, debug=False)

    xsT = nc.dram_tensor("xsT", [H, CT], f32r, kind="ExternalInput")
    gwP = nc.dram_tensor("gwP", [EPC, IT, 128, HC, 128], f32r, kind="ExternalInput")
    uwP = nc.dram_tensor("uwP", [EPC, IT, 128, HC, 128], f32r, kind="ExternalInput")
    dwT = nc.dram_tensor("dwT", [EPC, I, H], f32r, kind="ExternalInput")
    o = nc.dram_tensor("o", [CT, H], f32, kind="ExternalOutput")

    Ce = CT // EPC
    chunks = _row_chunks(Ce)

    with tile.TileContext(nc) as tc:
        with (
            tc.tile_pool(name="xs", bufs=1) as xsp,
            tc.tile_pool(name="wg", bufs=2) as wg,
            tc.tile_pool(name="wu", bufs=2) as wu,
            tc.tile_pool(name="wd", bufs=2) as wd,
            tc.tile_pool(name="hm", bufs=1) as hmp,
            tc.tile_pool(name="sg", bufs=2) as sgp,
            tc.tile_pool(name="ost", bufs=4) as ostp,
            tc.tile_pool(name="psum", bufs=6, space="PSUM") as psp,
        ):
            for e in range(EPC):
                for c0, cl in chunks:
                    base = e * Ce + c0
                    xst_full = xsp.tile([128, HC, 1152], f32r, tag="xs")
                    xst = xst_full[:, :, :cl]
                    for hcb in range(HC):
                        nc.sync.dma_start(
                            xst[:, hcb, :],
                            xsT.ap()[hcb * 128 : (hcb + 1) * 128, base : base + cl],
                        )
                    hm_full = hmp.tile([128, IC, 1152], f32r, tag="hm")
                    hm = hm_full[:, :, :cl]
                    for it in range(IT):
                        gw = wg.tile([128, HC, 128], f32r, tag="gw")
                        nc.gpsimd.dma_start(gw[:], gwP.ap()[e, it])
                        uw = wu.tile([128, HC, 128], f32r, tag="uw")
                        nc.gpsimd.dma_start(uw[:], uwP.ap()[e, it])
                        for r0, rl in _move_slices(cl):
                            pg_full = psp.tile([128, 512], f32, tag="ps")
                            pu_full = psp.tile([128, 512], f32, tag="ps")
                            pg = pg_full[:, :rl]
                            pu = pu_full[:, :rl]
                            for hc in range(HC):
                                nc.tensor.matmul(
                                    pg[:], gw[:, hc, :], xst[:, hc, r0 : r0 + rl],
                                    start=(hc == 0), stop=(hc == HC - 1),
                                )
                            for hc in range(HC):
                                nc.tensor.matmul(
                                    pu[:], uw[:, hc, :], xst[:, hc, r0 : r0 + rl],
                                    start=(hc == 0), stop=(hc == HC - 1),
                                )
                            sg_full = sgp.tile([128, 512], f32, tag="sg")
                            sg = sg_full[:, :rl]
                            nc.scalar.activation(
                                sg[:], pg[:], mybir.ActivationFunctionType.Silu
                            )
                            nc.vector.tensor_mul(hm[:, it, r0 : r0 + rl], sg[:], pu[:])

                    for hs in range(H // 512):
                        dw = wd.tile([128, IC, 512], f32r, tag="dw")
                        nc.sync.dma_start(
                            dw[:],
                            dwT.ap()[e][:, hs * 512 : (hs + 1) * 512].rearrange(
                                "(c p) h -> p c h", p=128
                            ),
                        )
                        for rt in range(cl // 128):
                            po = psp.tile([128, 512], f32, tag="ps")
                            for ic in range(IC):
                                nc.tensor.matmul(
                                    po[:], hm[:, ic, rt * 128 : (rt + 1) * 128],
                                    dw[:, ic, :], start=(ic == 0), stop=(ic == IC - 1),
                                )
                            ot = ostp.tile([128, 512], f32, tag="o")
                            nc.vector.tensor_copy(ot[:], po[:])
                            nc.scalar.dma_start(
                                o.ap()[
                                    base + rt * 128 : base + (rt + 1) * 128,
                                    hs * 512 : (hs + 1) * 512,
                                ],
                                ot[:],
                            )
    nc.compile()
    return nc


def _kernel_tf32(hidden_states, gate_weight, up_weight, down_weight, topk_idx, topk_weight):
    global _LAST_RESULTS
    from concourse.bass_utils import run_bass_kernel_spmd

    x = np.ascontiguousarray(hidden_states, dtype=np.float32).reshape(N, H)
    flat_expert = np.asarray(topk_idx).reshape(-1).astype(np.int64)
    flat_weight = np.asarray(topk_weight).reshape(-1).astype(np.float32)

    perm = np.argsort(flat_expert, kind="stable")
    tok_sorted = np.repeat(np.arange(N), K)[perm]
    sizes = np.bincount(flat_expert, minlength=E)
    offs = np.concatenate([[0], np.cumsum(sizes)])

    Ce = int(math.ceil(sizes.max() / 256) * 256)
    CT = EPC * Ce

    gw_all = np.asarray(gate_weight, dtype=np.float32)
    uw_all = np.asarray(up_weight, dtype=np.float32)
    dw_all = np.asarray(down_weight, dtype=np.float32)

    in_maps = []
    for m in range(NCORES):
        sl = slice(m * EPC, (m + 1) * EPC)
        xsT_m = np.zeros((H, CT), dtype=np.float32)
        for el in range(EPC):
            ex = m * EPC + el
            ids = tok_sorted[offs[ex] : offs[ex + 1]]
            xsT_m[:, el * Ce : el * Ce + len(ids)] = _round_tf32(x[ids]).T

        def pack_gu(w):  # w: (EPC, I, H)
            w4 = w.reshape(EPC, IT, 128, HC, 128)  # (e, it, il, hc, p)
            return _round_tf32(np.ascontiguousarray(w4.transpose(0, 1, 4, 3, 2)))

        in_maps.append(
            {
                "xsT": xsT_m,
                "gwP": pack_gu(gw_all[sl]),
                "uwP": pack_gu(uw_all[sl]),
                "dwT": _round_tf32(
                    np.ascontiguousarray(dw_all[sl].transpose(0, 2, 1))
                ),
            }
        )

    nc = _build_tf32(CT)
    res = run_bass_kernel_spmd(nc, in_maps, core_ids=list(range(NCORES)))
    _LAST_RESULTS = res

    o_sorted = np.empty((N * K, H), dtype=np.float32)
    for m in range(NCORES):
        om = res.results[m]["o"]
        for el in range(EPC):
            ex = m * EPC + el
            n_e = offs[ex + 1] - offs[ex]
            o_sorted[offs[ex] : offs[ex + 1]] = om[el * Ce : el * Ce + n_e]
    o_sorted *= flat_weight[perm][:, None]
    o_orig = np.empty_like(o_sorted)
    o_orig[perm] = o_sorted
    y = o_orig.reshape(N, K, H).sum(axis=1)
    return y.reshape(B, S, H).astype(np.float32)


def kernel(hidden_states, gate_weight, up_weight, down_weight, topk_idx, topk_weight):
    if VARIANT == "tf32":
        return _kernel_tf32(
            hidden_states, gate_weight, up_weight, down_weight, topk_idx, topk_weight
        )
    return _kernel_bf16x(
        hidden_states, gate_weight, up_weight, down_weight, topk_idx, topk_weight
    )


# revision 22
# speedup vs baseline: 1.1737x; 1.0112x over previous
"""Expert-parallel MoE grouped-experts kernel for 8 trn2 NeuronCores.

Contract: kernel(**inputs) takes FULL unsharded inputs, returns FULL output.

Strategy (expert-parallel, variant "bf16x" default):
  - Host: sort token-expert assignments by expert. Experts are ranked by
    size (desc) and dealt round-robin to cores: core c gets experts ranked
    c, 8+c, 16+c, 24+c. The SPMD program has 4 static block sizes
    Ce[b] = max size within rank octile b (so padding is the octile spread,
    ~1-2%, instead of global-max padding ~12%).
  - Device (SPMD x8, one static program): per block, per 1024-row chunk:
    grouped GEMMs with weights stationary and rows moving everywhere:
      pg/pu[i-tile, rows] = sum_hc gw/uw[hc, i-tile].T @ xs[hc, rows]
      hm[it] = silu(pg) * pu                       (bf16)
      oT[h-tile, rows] = sum_ic dw[ic, h-tile].T @ hm[ic]
    All matmuls bf16 at 1 cy/row, fp32 PSUM accumulation; LDWEIGHTS (128
    cols, FWL) hides behind 512-col matmuls.
  - Host: transpose oT back, scale by routing weights, scatter-add.

Older variants kept for fallback: "tf32" (f32r), "bf16" (uniform-Ce).
"""
import os
import sys

if "/opt/trn_rl_repo" not in sys.path:
    sys.path.insert(0, "/opt/trn_rl_repo")

import math
import numpy as np
import ml_dtypes

B, S, H, I, E, K = 4, 4096, 2048, 1024, 32, 4
N = B * S
NCORES = 8
EPC = E // NCORES  # experts per core (blocks)
HC = H // 128      # 16 h-chunks (gate/up contraction tiles)
IC = I // 128      # 8 i-chunks (down contraction tiles)
IT = I // 128      # 8 i-tiles
HT = H // 128      # 16 h-tiles (down output tiles)
CHUNK = 1152       # xs rows resident per chunk (tile allocation size)


def _even_split(total: int, cap: int, align: int = 16):
    """Split `total` into near-equal pieces of <= cap, multiples of `align`
    (except possibly the last). Avoids tiny tail pieces."""
    if total <= 0:
        return []
    n = math.ceil(total / cap)
    step = math.ceil(total / n / align) * align
    out = []
    r = 0
    while r < total:
        out.append((r, min(step, total - r)))
        r += out[-1][1]
    return out

VARIANT = os.environ.get("MOE_VARIANT", "bf16x")
_SIM_ACT = os.environ.get("MOE_SIM_ACT", "")  # e.g. "Sigmoid" for CoreSim checks

_LAST_RESULTS = None  # BassKernelResults of the most recent run (for test.py)


def _round_tf32(x: np.ndarray) -> np.ndarray:
    """Round f32 to tf32 (10-bit mantissa), round-to-nearest-even."""
    u = np.ascontiguousarray(x, dtype=np.float32).view(np.uint32).astype(np.uint64)
    u = u + 0x0FFF + ((u >> 13) & 1)
    u = (u & np.uint64(0xFFFFE000)).astype(np.uint32)
    return u.view(np.float32)


def _build_bf16x(ce_list):
    """bf16 exact-ish-size kernel: 4 expert blocks with static sizes ce_list."""
    import concourse.tile as tile
    import concourse.mybir as mybir
    from concourse import bacc

    bf16 = mybir.dt.bfloat16
    f32 = mybir.dt.float32

    CT = int(sum(ce_list))
    # flat chunk list: (block, c0, cl); xsC is chunk-major contiguous
    chunk_list = []
    for b in range(EPC):
        for c0, cl in _even_split(int(ce_list[b]), CHUNK):
            chunk_list.append((b, c0, cl))
    NCH = len(chunk_list)

    nc = bacc.Bacc("TRN2", target_bir_lowering=False, debug=False)

    xsT = nc.dram_tensor("xsT", [H, CT], bf16, kind="ExternalInput")
    gwP = nc.dram_tensor("gwP", [EPC, IT, 128, HC, 128], bf16, kind="ExternalInput")
    uwP = nc.dram_tensor("uwP", [EPC, IT, 128, HC, 128], bf16, kind="ExternalInput")
    dwP = nc.dram_tensor("dwP", [EPC, HT, 128, IC, 128], bf16, kind="ExternalInput")
    oT = nc.dram_tensor("oT", [H, CT], bf16, kind="ExternalOutput")

    with tile.TileContext(nc) as tc:
        with (
            tc.tile_pool(name="xs", bufs=3) as xsp,
            tc.tile_pool(name="hm", bufs=16) as hmp,
            tc.tile_pool(name="wg", bufs=2) as wgp,
            tc.tile_pool(name="wu", bufs=2) as wup,
            tc.tile_pool(name="wd", bufs=2) as wdp,
            tc.tile_pool(name="sg", bufs=2) as sgp,
            tc.tile_pool(name="ot", bufs=4) as otp,
            tc.tile_pool(name="psum", bufs=8, space="PSUM") as psp,
        ):
            bases = [int(sum(ce_list[:b])) for b in range(EPC)]

            for ci, (b, c0, cl) in enumerate(chunk_list):
                base = bases[b]
                xst_full = xsp.tile([128, HC, CHUNK], bf16, tag="xs", name="xst")
                xst = xst_full[:, :, :cl]
                nc.sync.dma_start(
                    xst,
                    xsT.ap()[:, base + c0 : base + c0 + cl].rearrange(
                        "(c p) r -> p c r", p=128
                    ),
                )
                if True:
                    slices = _even_split(cl, 512)
                    hms = []
                    for it in range(IT):
                        gw = wgp.tile([128, HC, 128], bf16, tag="gw")
                        nc.gpsimd.dma_start(gw[:], gwP.ap()[b, it])
                        uw = wup.tile([128, HC, 128], bf16, tag="uw")
                        nc.gpsimd.dma_start(uw[:], uwP.ap()[b, it])
                        hm_full = hmp.tile([128, CHUNK], bf16, tag="hm")
                        hm = hm_full[:, :cl]
                        hms.append(hm)
                        # one pg/pu PSUM bank per slice; stationary weight
                        # tiles are reused across all slices (slice-inner)
                        pgs = []
                        for r0, rl in slices:
                            pg_full = psp.tile([128, 512], f32, tag="ps")
                            pgs.append(pg_full[:, :rl])
                        for hc in range(HC):
                            for si, (r0, rl) in enumerate(slices):
                                nc.tensor.matmul(
                                    pgs[si], gw[:, hc, :], xst[:, hc, r0 : r0 + rl],
                                    start=(hc == 0), stop=(hc == HC - 1),
                                )
                        pus = []
                        for r0, rl in slices:
                            pu_full = psp.tile([128, 512], f32, tag="ps")
                            pus.append(pu_full[:, :rl])
                        for hc in range(HC):
                            for si, (r0, rl) in enumerate(slices):
                                nc.tensor.matmul(
                                    pus[si], uw[:, hc, :], xst[:, hc, r0 : r0 + rl],
                                    start=(hc == 0), stop=(hc == HC - 1),
                                )
                        act = getattr(
                            mybir.ActivationFunctionType, _SIM_ACT or "Silu"
                        )
                        for si, (r0, rl) in enumerate(slices):
                            sg_full = sgp.tile([128, 512], f32, tag="sg")
                            sg = sg_full[:, :rl]
                            nc.scalar.activation(sg, pgs[si], act)
                            nc.vector.tensor_mul(hm[:, r0 : r0 + rl], sg, pus[si])

                    for ht in range(HT):
                        dwt = wdp.tile([128, IC, 128], bf16, tag="dw")
                        nc.scalar.dma_start(dwt[:], dwP.ap()[b, ht])
                        # slice pairs: <=2 po banks live at a time
                        for p0 in range(0, len(slices), 2):
                            spair = slices[p0 : p0 + 2]
                            pos = []
                            for r0, rl in spair:
                                po_full = psp.tile([128, 512], f32, tag="ps")
                                pos.append(po_full[:, :rl])
                            for ic in range(IC):
                                for si, (r0, rl) in enumerate(spair):
                                    nc.tensor.matmul(
                                        pos[si], dwt[:, ic, :],
                                        hms[ic][:, r0 : r0 + rl],
                                        start=(ic == 0), stop=(ic == IC - 1),
                                    )
                            for si, (r0, rl) in enumerate(spair):
                                ot = otp.tile([128, 512], bf16, tag="ot")
                                ott = ot[:, :rl]
                                nc.vector.tensor_copy(ott, pos[si])
                                nc.scalar.dma_start(
                                    oT.ap()[
                                        ht * 128 : (ht + 1) * 128,
                                        base + c0 + r0 : base + c0 + r0 + rl,
                                    ],
                                    ott,
                                )
    nc.compile()
    return nc


def _pack_gu(w_bf):
    """(I, H) bf16 -> [IT, 128p(h%128), HC, 128(i%128)] contiguous."""
    w4 = w_bf.reshape(IT, 128, HC, 128)  # (it, il, hc, p)
    return np.ascontiguousarray(w4.transpose(0, 3, 2, 1))


def _pack_d(w_bf):
    """(H, I) bf16 -> [HT, 128p(i%128), IC, 128(h%128)] contiguous."""
    w4 = w_bf.reshape(HT, 128, IC, 128)  # (ht, hl, ic, p)
    return np.ascontiguousarray(w4.transpose(0, 3, 2, 1))


def _kernel_bf16x(hidden_states, gate_weight, up_weight, down_weight, topk_idx, topk_weight):
    global _LAST_RESULTS
    from concourse.bass_utils import run_bass_kernel_spmd

    bf16 = ml_dtypes.bfloat16

    x = np.ascontiguousarray(hidden_states, dtype=np.float32).reshape(N, H)
    flat_expert = np.asarray(topk_idx).reshape(-1).astype(np.int64)
    flat_weight = np.asarray(topk_weight).reshape(-1).astype(np.float32)

    perm = np.argsort(flat_expert, kind="stable")
    tok_sorted = np.repeat(np.arange(N), K)[perm]
    sizes = np.bincount(flat_expert, minlength=E)
    offs = np.concatenate([[0], np.cumsum(sizes)])

    order = np.argsort(-sizes, kind="stable")  # experts by size desc
    # block b on core c processes expert order[b*NCORES + c]
    ce_list = [
        int(math.ceil(max(int(sizes[order[b * NCORES]]), 16) / 16) * 16)
        for b in range(EPC)
    ]
    bases = np.concatenate([[0], np.cumsum(ce_list)])
    CT = int(bases[-1])
    chunk_list = []
    for b in range(EPC):
        for c0, cl in _even_split(int(ce_list[b]), CHUNK):
            chunk_list.append((b, c0, cl))
    NCH = len(chunk_list)

    xs_all = x[tok_sorted].astype(bf16)  # (N*K, H) expert-contiguous rows
    gw_bf = np.asarray(gate_weight).astype(bf16)
    uw_bf = np.asarray(up_weight).astype(bf16)
    dw_bf = np.asarray(down_weight).astype(bf16)

    in_maps = []
    for c in range(NCORES):
        xsT_m = np.zeros((H, CT), dtype=bf16)
        gw_l, uw_l, dw_l = [], [], []
        for b in range(EPC):
            e = int(order[b * NCORES + c])
            n_e = int(sizes[e])
            xsT_m[:, bases[b] : bases[b] + n_e] = xs_all[offs[e] : offs[e + 1]].T
            gw_l.append(_pack_gu(gw_bf[e]))
            uw_l.append(_pack_gu(uw_bf[e]))
            dw_l.append(_pack_d(dw_bf[e]))
        in_maps.append(
            {
                "xsT": xsT_m,
                "gwP": np.stack(gw_l),
                "uwP": np.stack(uw_l),
                "dwP": np.stack(dw_l),
            }
        )

    nc = _build_bf16x(ce_list)
    res = run_bass_kernel_spmd(nc, in_maps, core_ids=list(range(NCORES)))
    _LAST_RESULTS = res

    # combine: weighted scatter-add back to token order
    o_sorted = np.empty((N * K, H), dtype=np.float32)
    for c in range(NCORES):
        om = res.results[c]["oT"]  # (H, CT) bf16
        for b in range(EPC):
            e = int(order[b * NCORES + c])
            n_e = int(sizes[e])
            o_sorted[offs[e] : offs[e + 1]] = (
                om[:, bases[b] : bases[b] + n_e].T.astype(np.float32)
            )
    o_sorted *= flat_weight[perm][:, None]
    o_orig = np.empty_like(o_sorted)
    o_orig[perm] = o_sorted
    y = o_orig.reshape(N, K, H).sum(axis=1)
    return y.reshape(B, S, H).astype(np.float32)


# ---------------------------------------------------------------------------
# Legacy variants (tf32 / bf16 uniform-Ce), kept as fallback.
# ---------------------------------------------------------------------------


def _row_chunks(ce: int, chunk: int = 1152):
    out = []
    r = 0
    while r < ce:
        out.append((r, min(chunk, ce - r)))
        r += out[-1][1]
    return out


def _move_slices(length: int):
    out = []
    r = 0
    while r < length:
        rem = length - r
        if rem <= 512:
            s = rem
        elif rem - 384 >= 256:
            s = 384
        else:
            s = 512
        out.append((r, s))
        r += s
    return out


def _build_tf32(CT: int):
    import concourse.tile as tile
    import concourse.mybir as mybir
    from concourse import bacc

    f32 = mybir.dt.float32
    f32r = mybir.dt.float32r

    nc = bacc.Bacc("TRN2", target_bir_lowering=False, debug=False)

    xsT = nc.dram_tensor("xsT", [H, CT], f32r, kind="ExternalInput")
    gwP = nc.dram_tensor("gwP", [EPC, IT, 128, HC, 128], f32r, kind="ExternalInput")
    uwP = nc.dram_tensor("uwP", [EPC, IT, 128, HC, 128], f32r, kind="ExternalInput")
    dwT = nc.dram_tensor("dwT", [EPC, I, H], f32r, kind="ExternalInput")
    o = nc.dram_tensor("o", [CT, H], f32, kind="ExternalOutput")

    Ce = CT // EPC
    chunks = _row_chunks(Ce)

    with tile.TileContext(nc) as tc:
        with (
            tc.tile_pool(name="xs", bufs=1) as xsp,
            tc.tile_pool(name="wg", bufs=2) as wg,
            tc.tile_pool(name="wu", bufs=2) as wu,
            tc.tile_pool(name="wd", bufs=2) as wd,
            tc.tile_pool(name="hm", bufs=1) as hmp,
            tc.tile_pool(name="sg", bufs=2) as sgp,
            tc.tile_pool(name="ost", bufs=4) as ostp,
            tc.tile_pool(name="psum", bufs=6, space="PSUM") as psp,
        ):
            for e in range(EPC):
                for c0, cl in chunks:
                    base = e * Ce + c0
                    xst_full = xsp.tile([128, HC, 1152], f32r, tag="xs")
                    xst = xst_full[:, :, :cl]
                    for hcb in range(HC):
                        nc.sync.dma_start(
                            xst[:, hcb, :],
                            xsT.ap()[hcb * 128 : (hcb + 1) * 128, base : base + cl],
                        )
                    hm_full = hmp.tile([128, IC, 1152], f32r, tag="hm")
                    hm = hm_full[:, :, :cl]
                    for it in range(IT):
                        gw = wg.tile([128, HC, 128], f32r, tag="gw")
                        nc.gpsimd.dma_start(gw[:], gwP.ap()[e, it])
                        uw = wu.tile([128, HC, 128], f32r, tag="uw")
                        nc.gpsimd.dma_start(uw[:], uwP.ap()[e, it])
                        for r0, rl in _move_slices(cl):
                            pg_full = psp.tile([128, 512], f32, tag="ps")
                            pu_full = psp.tile([128, 512], f32, tag="ps")
                            pg = pg_full[:, :rl]
                            pu = pu_full[:, :rl]
                            for hc in range(HC):
                                nc.tensor.matmul(
                                    pg[:], gw[:, hc, :], xst[:, hc, r0 : r0 + rl],
                                    start=(hc == 0), stop=(hc == HC - 1),
                                )
                            for hc in range(HC):
                                nc.tensor.matmul(
                                    pu[:], uw[:, hc, :], xst[:, hc, r0 : r0 + rl],
                                    start=(hc == 0), stop=(hc == HC - 1),
                                )
                            sg_full = sgp.tile([128, 512], f32, tag="sg")
                            sg = sg_full[:, :rl]
                            nc.scalar.activation(
                                sg[:], pg[:], mybir.ActivationFunctionType.Silu
                            )
                            nc.vector.tensor_mul(hm[:, it, r0 : r0 + rl], sg[:], pu[:])

                    for hs in range(H // 512):
                        dw = wd.tile([128, IC, 512], f32r, tag="dw")
                        nc.sync.dma_start(
                            dw[:],
                            dwT.ap()[e][:, hs * 512 : (hs + 1) * 512].rearrange(
                                "(c p) h -> p c h", p=128
                            ),
                        )
                        for rt in range(cl // 128):
                            po = psp.tile([128, 512], f32, tag="ps")
                            for ic in range(IC):
                                nc.tensor.matmul(
                                    po[:], hm[:, ic, rt * 128 : (rt + 1) * 128],
                                    dw[:, ic, :], start=(ic == 0), stop=(ic == IC - 1),
                                )
                            ot = ostp.tile([128, 512], f32, tag="o")
                            nc.vector.tensor_copy(ot[:], po[:])
                            nc.scalar.dma_start(
                                o.ap()[
                                    base + rt * 128 : base + (rt + 1) * 128,
                                    hs * 512 : (hs + 1) * 512,
                                ],
                                ot[:],
                            )
    nc.compile()
    return nc


def _kernel_tf32(hidden_states, gate_weight, up_weight, down_weight, topk_idx, topk_weight):
    global _LAST_RESULTS
    from concourse.bass_utils import run_bass_kernel_spmd

    x = np.ascontiguousarray(hidden_states, dtype=np.float32).reshape(N, H)
    flat_expert = np.asarray(topk_idx).reshape(-1).astype(np.int64)
    flat_weight = np.asarray(topk_weight).reshape(-1).astype(np.float32)

    perm = np.argsort(flat_expert, kind="stable")
    tok_sorted = np.repeat(np.arange(N), K)[perm]
    sizes = np.bincount(flat_expert, minlength=E)
    offs = np.concatenate([[0], np.cumsum(sizes)])

    Ce = int(math.ceil(sizes.max() / 256) * 256)
    CT = EPC * Ce

    gw_all = np.asarray(gate_weight, dtype=np.float32)
    uw_all = np.asarray(up_weight, dtype=np.float32)
    dw_all = np.asarray(down_weight, dtype=np.float32)

    in_maps = []
    for m in range(NCORES):
        sl = slice(m * EPC, (m + 1) * EPC)
        xsT_m = np.zeros((H, CT), dtype=np.float32)
        for el in range(EPC):
            ex = m * EPC + el
            ids = tok_sorted[offs[ex] : offs[ex + 1]]
            xsT_m[:, el * Ce : el * Ce + len(ids)] = _round_tf32(x[ids]).T

        def pack_gu(w):  # w: (EPC, I, H)
            w4 = w.reshape(EPC, IT, 128, HC, 128)  # (e, it, il, hc, p)
            return _round_tf32(np.ascontiguousarray(w4.transpose(0, 1, 4, 3, 2)))

        in_maps.append(
            {
                "xsT": xsT_m,
                "gwP": pack_gu(gw_all[sl]),
                "uwP": pack_gu(uw_all[sl]),
                "dwT": _round_tf32(
                    np.ascontiguousarray(dw_all[sl].transpose(0, 2, 1))
                ),
            }
        )

    nc = _build_tf32(CT)
    res = run_bass_kernel_spmd(nc, in_maps, core_ids=list(range(NCORES)))
    _LAST_RESULTS = res

    o_sorted = np.empty((N * K, H), dtype=np.float32)
    for m in range(NCORES):
        om = res.results[m]["o"]
        for el in range(EPC):
            ex = m * EPC + el
            n_e = offs[ex + 1] - offs[ex]
            o_sorted[offs[ex] : offs[ex + 1]] = om[el * Ce : el * Ce + n_e]
    o_sorted *= flat_weight[perm][:, None]
    o_orig = np.empty_like(o_sorted)
    o_orig[perm] = o_sorted
    y = o_orig.reshape(N, K, H).sum(axis=1)
    return y.reshape(B, S, H).astype(np.float32)


def kernel(hidden_states, gate_weight, up_weight, down_weight, topk_idx, topk_weight):
    if VARIANT == "tf32":
        return _kernel_tf32(
            hidden_states, gate_weight, up_weight, down_weight, topk_idx, topk_weight
        )
    return _kernel_bf16x(
        hidden_states, gate_weight, up_weight, down_weight, topk_idx, topk_weight
    )


# revision 24
# speedup vs baseline: 1.1973x; 1.0201x over previous
"""Expert-parallel MoE grouped-experts kernel for 8 trn2 NeuronCores.

Contract: kernel(**inputs) takes FULL unsharded inputs, returns FULL output.

Strategy (expert-parallel, variant "bf16x" default):
  - Host: sort token-expert assignments by expert. Experts are ranked by
    size (desc) and dealt round-robin to cores: core c gets experts ranked
    c, 8+c, 16+c, 24+c. The SPMD program has 4 static block sizes
    Ce[b] = max size within rank octile b (so padding is the octile spread,
    ~1-2%, instead of global-max padding ~12%).
  - Device (SPMD x8, one static program): per block, per 1024-row chunk:
    grouped GEMMs with weights stationary and rows moving everywhere:
      pg/pu[i-tile, rows] = sum_hc gw/uw[hc, i-tile].T @ xs[hc, rows]
      hm[it] = silu(pg) * pu                       (bf16)
      oT[h-tile, rows] = sum_ic dw[ic, h-tile].T @ hm[ic]
    All matmuls bf16 at 1 cy/row, fp32 PSUM accumulation; LDWEIGHTS (128
    cols, FWL) hides behind 512-col matmuls.
  - Host: transpose oT back, scale by routing weights, scatter-add.

Older variants kept for fallback: "tf32" (f32r), "bf16" (uniform-Ce).
"""
import os
import sys

if "/opt/trn_rl_repo" not in sys.path:
    sys.path.insert(0, "/opt/trn_rl_repo")

import math
import numpy as np
import ml_dtypes

B, S, H, I, E, K = 4, 4096, 2048, 1024, 32, 4
N = B * S
NCORES = 8
EPC = E // NCORES  # experts per core (blocks)
HC = H // 128      # 16 h-chunks (gate/up contraction tiles)
IC = I // 128      # 8 i-chunks (down contraction tiles)
IT = I // 128      # 8 i-tiles
HT = H // 128      # 16 h-tiles (down output tiles)
CHUNK = 1152       # xs rows resident per chunk (tile allocation size)


def _even_split(total: int, cap: int, align: int = 16):
    """Split `total` into near-equal pieces of <= cap, multiples of `align`
    (except possibly the last). Avoids tiny tail pieces."""
    if total <= 0:
        return []
    n = math.ceil(total / cap)
    step = math.ceil(total / n / align) * align
    out = []
    r = 0
    while r < total:
        out.append((r, min(step, total - r)))
        r += out[-1][1]
    return out

VARIANT = os.environ.get("MOE_VARIANT", "bf16x")
_SIM_ACT = os.environ.get("MOE_SIM_ACT", "")  # e.g. "Sigmoid" for CoreSim checks

_LAST_RESULTS = None  # BassKernelResults of the most recent run (for test.py)


def _round_tf32(x: np.ndarray) -> np.ndarray:
    """Round f32 to tf32 (10-bit mantissa), round-to-nearest-even."""
    u = np.ascontiguousarray(x, dtype=np.float32).view(np.uint32).astype(np.uint64)
    u = u + 0x0FFF + ((u >> 13) & 1)
    u = (u & np.uint64(0xFFFFE000)).astype(np.uint32)
    return u.view(np.float32)


def _build_bf16x(ce_list):
    """bf16 exact-ish-size kernel: 4 expert blocks with static sizes ce_list."""
    import concourse.tile as tile
    import concourse.mybir as mybir
    from concourse import bacc

    bf16 = mybir.dt.bfloat16
    f32 = mybir.dt.float32

    CT = int(sum(ce_list))
    # flat chunk list: (block, c0, cl); xsC is chunk-major contiguous
    chunk_list = []
    for b in range(EPC):
        for c0, cl in _even_split(int(ce_list[b]), CHUNK):
            chunk_list.append((b, c0, cl))
    NCH = len(chunk_list)

    nc = bacc.Bacc("TRN2", target_bir_lowering=False, debug=False)

    xsT = nc.dram_tensor("xsT", [H, CT], bf16, kind="ExternalInput")
    gwP = nc.dram_tensor("gwP", [EPC, IT, 128, HC, 128], bf16, kind="ExternalInput")
    uwP = nc.dram_tensor("uwP", [EPC, IT, 128, HC, 128], bf16, kind="ExternalInput")
    dwP = nc.dram_tensor("dwP", [EPC, HT, 128, IC, 128], bf16, kind="ExternalInput")
    oT = nc.dram_tensor("oT", [H, CT], bf16, kind="ExternalOutput")

    with tile.TileContext(nc) as tc:
        with (
            tc.tile_pool(name="xs", bufs=3) as xsp,
            tc.tile_pool(name="hm", bufs=16) as hmp,
            tc.tile_pool(name="wg", bufs=2) as wgp,
            tc.tile_pool(name="wu", bufs=2) as wup,
            tc.tile_pool(name="wd", bufs=2) as wdp,
            tc.tile_pool(name="sg", bufs=2) as sgp,
            tc.tile_pool(name="ot", bufs=4) as otp,
            tc.tile_pool(name="psum", bufs=8, space="PSUM") as psp,
        ):
            bases = [int(sum(ce_list[:b])) for b in range(EPC)]

            for ci, (b, c0, cl) in enumerate(chunk_list):
                base = bases[b]
                xst_full = xsp.tile([128, HC, CHUNK], bf16, tag="xs", name="xst")
                xst = xst_full[:, :, :cl]
                slices = _even_split(cl, 512)
                for r0, rl in slices:
                    nc.sync.dma_start(
                        xst[:, :, r0 : r0 + rl],
                        xsT.ap()[
                            :, base + c0 + r0 : base + c0 + r0 + rl
                        ].rearrange("(c p) r -> p c r", p=128),
                    )
                if True:
                    hms = []
                    for it in range(IT):
                        gw = wgp.tile([128, HC, 128], bf16, tag="gw")
                        nc.gpsimd.dma_start(gw[:], gwP.ap()[b, it])
                        uw = wup.tile([128, HC, 128], bf16, tag="uw")
                        nc.gpsimd.dma_start(uw[:], uwP.ap()[b, it])
                        hm_full = hmp.tile([128, CHUNK], bf16, tag="hm")
                        hm = hm_full[:, :cl]
                        hms.append(hm)
                        # one pg/pu PSUM bank per slice; stationary weight
                        # tiles are reused across all slices (slice-inner)
                        pgs = []
                        for r0, rl in slices:
                            pg_full = psp.tile([128, 512], f32, tag="ps")
                            pgs.append(pg_full[:, :rl])
                        for hc in range(HC):
                            for si, (r0, rl) in enumerate(slices):
                                nc.tensor.matmul(
                                    pgs[si], gw[:, hc, :], xst[:, hc, r0 : r0 + rl],
                                    start=(hc == 0), stop=(hc == HC - 1),
                                )
                        pus = []
                        for r0, rl in slices:
                            pu_full = psp.tile([128, 512], f32, tag="ps")
                            pus.append(pu_full[:, :rl])
                        for hc in range(HC):
                            for si, (r0, rl) in enumerate(slices):
                                nc.tensor.matmul(
                                    pus[si], uw[:, hc, :], xst[:, hc, r0 : r0 + rl],
                                    start=(hc == 0), stop=(hc == HC - 1),
                                )
                        act = getattr(
                            mybir.ActivationFunctionType, _SIM_ACT or "Silu"
                        )
                        for si, (r0, rl) in enumerate(slices):
                            sg_full = sgp.tile([128, 512], f32, tag="sg")
                            sg = sg_full[:, :rl]
                            nc.scalar.activation(sg, pgs[si], act)
                            nc.vector.tensor_mul(hm[:, r0 : r0 + rl], sg, pus[si])

                    for ht in range(HT):
                        dwt = wdp.tile([128, IC, 128], bf16, tag="dw")
                        nc.gpsimd.dma_start(dwt[:], dwP.ap()[b, ht])
                        # slice pairs: <=2 po banks live at a time
                        for p0 in range(0, len(slices), 2):
                            spair = slices[p0 : p0 + 2]
                            pos = []
                            for r0, rl in spair:
                                po_full = psp.tile([128, 512], f32, tag="ps")
                                pos.append(po_full[:, :rl])
                            for ic in range(IC):
                                for si, (r0, rl) in enumerate(spair):
                                    nc.tensor.matmul(
                                        pos[si], dwt[:, ic, :],
                                        hms[ic][:, r0 : r0 + rl],
                                        start=(ic == 0), stop=(ic == IC - 1),
                                    )
                            for si, (r0, rl) in enumerate(spair):
                                ot = otp.tile([128, 512], bf16, tag="ot")
                                ott = ot[:, :rl]
                                nc.vector.tensor_copy(ott, pos[si])
                                nc.scalar.dma_start(
                                    oT.ap()[
                                        ht * 128 : (ht + 1) * 128,
                                        base + c0 + r0 : base + c0 + r0 + rl,
                                    ],
                                    ott,
                                )
    nc.compile()
    return nc


def _pack_gu(w_bf):
    """(I, H) bf16 -> [IT, 128p(h%128), HC, 128(i%128)] contiguous."""
    w4 = w_bf.reshape(IT, 128, HC, 128)  # (it, il, hc, p)
    return np.ascontiguousarray(w4.transpose(0, 3, 2, 1))


def _pack_d(w_bf):
    """(H, I) bf16 -> [HT, 128p(i%128), IC, 128(h%128)] contiguous."""
    w4 = w_bf.reshape(HT, 128, IC, 128)  # (ht, hl, ic, p)
    return np.ascontiguousarray(w4.transpose(0, 3, 2, 1))


def _kernel_bf16x(hidden_states, gate_weight, up_weight, down_weight, topk_idx, topk_weight):
    global _LAST_RESULTS
    from concourse.bass_utils import run_bass_kernel_spmd

    bf16 = ml_dtypes.bfloat16

    x = np.ascontiguousarray(hidden_states, dtype=np.float32).reshape(N, H)
    flat_expert = np.asarray(topk_idx).reshape(-1).astype(np.int64)
    flat_weight = np.asarray(topk_weight).reshape(-1).astype(np.float32)

    perm = np.argsort(flat_expert, kind="stable")
    tok_sorted = np.repeat(np.arange(N), K)[perm]
    sizes = np.bincount(flat_expert, minlength=E)
    offs = np.concatenate([[0], np.cumsum(sizes)])

    order = np.argsort(-sizes, kind="stable")  # experts by size desc
    # block b on core c processes expert order[b*NCORES + c]
    ce_list = [
        int(math.ceil(max(int(sizes[order[b * NCORES]]), 16) / 16) * 16)
        for b in range(EPC)
    ]
    bases = np.concatenate([[0], np.cumsum(ce_list)])
    CT = int(bases[-1])
    chunk_list = []
    for b in range(EPC):
        for c0, cl in _even_split(int(ce_list[b]), CHUNK):
            chunk_list.append((b, c0, cl))
    NCH = len(chunk_list)

    xs_all = x[tok_sorted].astype(bf16)  # (N*K, H) expert-contiguous rows
    gw_bf = np.asarray(gate_weight).astype(bf16)
    uw_bf = np.asarray(up_weight).astype(bf16)
    dw_bf = np.asarray(down_weight).astype(bf16)

    in_maps = []
    for c in range(NCORES):
        xsT_m = np.zeros((H, CT), dtype=bf16)
        gw_l, uw_l, dw_l = [], [], []
        for b in range(EPC):
            e = int(order[b * NCORES + c])
            n_e = int(sizes[e])
            xsT_m[:, bases[b] : bases[b] + n_e] = xs_all[offs[e] : offs[e + 1]].T
            gw_l.append(_pack_gu(gw_bf[e]))
            uw_l.append(_pack_gu(uw_bf[e]))
            dw_l.append(_pack_d(dw_bf[e]))
        in_maps.append(
            {
                "xsT": xsT_m,
                "gwP": np.stack(gw_l),
                "uwP": np.stack(uw_l),
                "dwP": np.stack(dw_l),
            }
        )

    nc = _build_bf16x(ce_list)
    res = run_bass_kernel_spmd(nc, in_maps, core_ids=list(range(NCORES)))
    _LAST_RESULTS = res

    # combine: weighted scatter-add back to token order
    o_sorted = np.empty((N * K, H), dtype=np.float32)
    for c in range(NCORES):
        om = res.results[c]["oT"]  # (H, CT) bf16
        for b in range(EPC):
            e = int(order[b * NCORES + c])
            n_e = int(sizes[e])
            o_sorted[offs[e] : offs[e + 1]] = (
                om[:, bases[b] : bases[b] + n_e].T.astype(np.float32)
            )
    o_sorted *= flat_weight[perm][:, None]
    o_orig = np.empty_like(o_sorted)
    o_orig[perm] = o_sorted
    y = o_orig.reshape(N, K, H).sum(axis=1)
    return y.reshape(B, S, H).astype(np.float32)


# ---------------------------------------------------------------------------
# Legacy variants (tf32 / bf16 uniform-Ce), kept as fallback.
# ---------------------------------------------------------------------------


def _row_chunks(ce: int, chunk: int = 1152):
    out = []
    r = 0
    while r < ce:
        out.append((r, min(chunk, ce - r)))
        r += out[-1][1]
    return out


def _move_slices(length: int):
    out = []
    r = 0
    while r < length:
        rem = length - r
        if rem <= 512:
            s = rem
        elif rem - 384 >= 256:
            s = 384
        else:
            s = 512
        out.append((r, s))
        r += s
    return out


def _build_tf32(CT: int):
    import concourse.tile as tile
    import concourse.mybir as mybir
    from concourse import bacc

    f32 = mybir.dt.float32
    f32r = mybir.dt.float32r

    nc = bacc.Bacc("TRN2", target_bir_lowering=False, debug=False)

    xsT = nc.dram_tensor("xsT", [H, CT], f32r, kind="ExternalInput")
    gwP = nc.dram_tensor("gwP", [EPC, IT, 128, HC, 128], f32r, kind="ExternalInput")
    uwP = nc.dram_tensor("uwP", [EPC, IT, 128, HC, 128], f32r, kind="ExternalInput")
    dwT = nc.dram_tensor("dwT", [EPC, I, H], f32r, kind="ExternalInput")
    o = nc.dram_tensor("o", [CT, H], f32, kind="ExternalOutput")

    Ce = CT // EPC
    chunks = _row_chunks(Ce)

    with tile.TileContext(nc) as tc:
        with (
            tc.tile_pool(name="xs", bufs=1) as xsp,
            tc.tile_pool(name="wg", bufs=2) as wg,
            tc.tile_pool(name="wu", bufs=2) as wu,
            tc.tile_pool(name="wd", bufs=2) as wd,
            tc.tile_pool(name="hm", bufs=1) as hmp,
            tc.tile_pool(name="sg", bufs=2) as sgp,
            tc.tile_pool(name="ost", bufs=4) as ostp,
            tc.tile_pool(name="psum", bufs=6, space="PSUM") as psp,
        ):
            for e in range(EPC):
                for c0, cl in chunks:
                    base = e * Ce + c0
                    xst_full = xsp.tile([128, HC, 1152], f32r, tag="xs")
                    xst = xst_full[:, :, :cl]
                    for hcb in range(HC):
                        nc.sync.dma_start(
                            xst[:, hcb, :],
                            xsT.ap()[hcb * 128 : (hcb + 1) * 128, base : base + cl],
                        )
                    hm_full = hmp.tile([128, IC, 1152], f32r, tag="hm")
                    hm = hm_full[:, :, :cl]
                    for it in range(IT):
                        gw = wg.tile([128, HC, 128], f32r, tag="gw")
                        nc.gpsimd.dma_start(gw[:], gwP.ap()[e, it])
                        uw = wu.tile([128, HC, 128], f32r, tag="uw")
                        nc.gpsimd.dma_start(uw[:], uwP.ap()[e, it])
                        for r0, rl in _move_slices(cl):
                            pg_full = psp.tile([128, 512], f32, tag="ps")
                            pu_full = psp.tile([128, 512], f32, tag="ps")
                            pg = pg_full[:, :rl]
                            pu = pu_full[:, :rl]
                            for hc in range(HC):
                                nc.tensor.matmul(
                                    pg[:], gw[:, hc, :], xst[:, hc, r0 : r0 + rl],
                                    start=(hc == 0), stop=(hc == HC - 1),
                                )
                            for hc in range(HC):
                                nc.tensor.matmul(
                                    pu[:], uw[:, hc, :], xst[:, hc, r0 : r0 + rl],
                                    start=(hc == 0), stop=(hc == HC - 1),
                                )
                            sg_full = sgp.tile([128, 512], f32, tag="sg")
                            sg = sg_full[:, :rl]
                            nc.scalar.activation(
                                sg[:], pg[:], mybir.ActivationFunctionType.Silu
                            )
                            nc.vector.tensor_mul(hm[:, it, r0 : r0 + rl], sg[:], pu[:])

                    for hs in range(H // 512):
                        dw = wd.tile([128, IC, 512], f32r, tag="dw")
                        nc.sync.dma_start(
                            dw[:],
                            dwT.ap()[e][:, hs * 512 : (hs + 1) * 512].rearrange(
                                "(c p) h -> p c h", p=128
                            ),
                        )
                        for rt in range(cl // 128):
                            po = psp.tile([128, 512], f32, tag="ps")
                            for ic in range(IC):
                                nc.tensor.matmul(
                                    po[:], hm[:, ic, rt * 128 : (rt + 1) * 128],
                                    dw[:, ic, :], start=(ic == 0), stop=(ic == IC - 1),
                                )
                            ot = ostp.tile([128, 512], f32, tag="o")
                            nc.vector.tensor_copy(ot[:], po[:])
                            nc.scalar.dma_start(
                                o.ap()[
                                    base + rt * 128 : base + (rt + 1) * 128,
                                    hs * 512 : (hs + 1) * 512,
                                ],
                                ot[:],
                            )
    nc.compile()
    return nc


def _kernel_tf32(hidden_states, gate_weight, up_weight, down_weight, topk_idx, topk_weight):
    global _LAST_RESULTS
    from concourse.bass_utils import run_bass_kernel_spmd

    x = np.ascontiguousarray(hidden_states, dtype=np.float32).reshape(N, H)
    flat_expert = np.asarray(topk_idx).reshape(-1).astype(np.int64)
    flat_weight = np.asarray(topk_weight).reshape(-1).astype(np.float32)

    perm = np.argsort(flat_expert, kind="stable")
    tok_sorted = np.repeat(np.arange(N), K)[perm]
    sizes = np.bincount(flat_expert, minlength=E)
    offs = np.concatenate([[0], np.cumsum(sizes)])

    Ce = int(math.ceil(sizes.max() / 256) * 256)
    CT = EPC * Ce

    gw_all = np.asarray(gate_weight, dtype=np.float32)
    uw_all = np.asarray(up_weight, dtype=np.float32)
    dw_all = np.asarray(down_weight, dtype=np.float32)

    in_maps = []
    for m in range(NCORES):
        sl = slice(m * EPC, (m + 1) * EPC)
        xsT_m = np.zeros((H, CT), dtype=np.float32)
        for el in range(EPC):
            ex = m * EPC + el
            ids = tok_sorted[offs[ex] : offs[ex + 1]]
            xsT_m[:, el * Ce : el * Ce + len(ids)] = _round_tf32(x[ids]).T

        def pack_gu(w):  # w: (EPC, I, H)
            w4 = w.reshape(EPC, IT, 128, HC, 128)  # (e, it, il, hc, p)
            return _round_tf32(np.ascontiguousarray(w4.transpose(0, 1, 4, 3, 2)))

        in_maps.append(
            {
                "xsT": xsT_m,
                "gwP": pack_gu(gw_all[sl]),
                "uwP": pack_gu(uw_all[sl]),
                "dwT": _round_tf32(
                    np.ascontiguousarray(dw_all[sl].transpose(0, 2, 1))
                ),
            }
        )

    nc = _build_tf32(CT)
    res = run_bass_kernel_spmd(nc, in_maps, core_ids=list(range(NCORES)))
    _LAST_RESULTS = res

    o_sorted = np.empty((N * K, H), dtype=np.float32)
    for m in range(NCORES):
        om = res.results[m]["o"]
        for el in range(EPC):
            ex = m * EPC + el
            n_e = offs[ex + 1] - offs[ex]
            o_sorted[offs[ex] : offs[ex + 1]] = om[el * Ce : el * Ce + n_e]
    o_sorted *= flat_weight[perm][:, None]
    o_orig = np.empty_like(o_sorted)
    o_orig[perm] = o_sorted
    y = o_orig.reshape(N, K, H).sum(axis=1)
    return y.reshape(B, S, H).astype(np.float32)


def kernel(hidden_states, gate_weight, up_weight, down_weight, topk_idx, topk_weight):
    if VARIANT == "tf32":
        return _kernel_tf32(
            hidden_states, gate_weight, up_weight, down_weight, topk_idx, topk_weight
        )
    return _kernel_bf16x(
        hidden_states, gate_weight, up_weight, down_weight, topk_idx, topk_weight
    )
